# revision 2
# baseline (speedup 1.0000x reference)
"""Trainium2 Bass kernel for nn_NEURAL_PYSCF_WF (neural wavefunction).

reference:
  mo   = einsum('ben,mn->bem', ao, mo_weight)          # [B, 32, 128]
  sub  = mo[:, cfg[:,:,None], cfg[:,None,:]]           # [B, 128, 16, 16]
  dets = det(sub)                                      # [B, 128]
  out  = dets @ ci_weight.T                            # [B, 1]

Config indices are < 32, so only mo[:, :, :32] matters.

V2 strategy (8 NeuronCores, data-parallel over B=8192). Per core (1024
rows = 8 b-tiles of 128 walker-partitions):
  phase A (per b-tile): host supplies ao^T [nao, bc*ne]; 32 tiles of
    128 be-rows each: matmul with the ao^T tile as the stationary
    operand (lhsT=[n, be], rhs=w32t [n, 32]) -> PSUM [be, 32m] -> ACT
    copy -> SBUF -> SBUF->SBUF DMA regroup into M_em [128b, (e, m32)].
    No PE pre-transpose, no DRAM scratch roundtrip.
  phase B (per b-tile, chunks of CT=8 configs):
    g1 (GPSIMD ap_gather d=16): config rows -> R [c, i, m32]
    tr (ACT strided copy):      R -> Rt [c, m32, i16]
    g2 (GPSIMD ap_gather d=16): config cols -> S [c, j, i] written
       into a 4-chunk group tile (32 configs / DVE op via 4D APs)
    pivot-free LU on DVE over [g, c, j, i]; reciprocal clamped to
    +-1e6; det = prod(diag) via product tree.
  out[b] = sum_c ci[c] * det[b, c]  (TT mult + reduce).
"""

from contextlib import ExitStack

import numpy as np

import concourse.bass as bass
import concourse.bacc as bacc
import concourse.mybir as mybir
import concourse.tile as tile
from concourse.bass_utils import run_bass_kernel_spmd

F32 = mybir.dt.float32
I16 = mybir.dt.int16
AX = mybir.AxisListType
OP = mybir.AluOpType

B = 8192
NE = 32      # electrons (and the max config index)
NAO = 128
K = 16       # config size
NCONF = 128
NCORES = 8
BC = B // NCORES
RCLAMP = 1e6
CT = 8       # configs per gather chunk
GC = 4       # chunks per LU op-group (GC*CT = 32 configs per DVE op)


def wrap_idx(idx: np.ndarray) -> np.ndarray:
    """Wrap a flat index list into ap_gather's [128, n/16] layout."""
    n = idx.shape[0]
    assert n % 16 == 0
    w = idx.reshape(n // 16, 16).T.astype(np.int16)
    return np.tile(w, (8, 1))


def build_gidx1(cfg: np.ndarray) -> np.ndarray:
    """Stage-1 indices per chunk: (c, i, h) -> cfg[c,i]*2 + h (d=16)."""
    cols = []
    for ch in range(NCONF // CT):
        sl = cfg[ch * CT:(ch + 1) * CT]                    # [CT, 16]
        idx = (sl[:, :, None].astype(np.int64) * 2
               + np.arange(2)[None, None, :]).reshape(-1)  # CT*K*2
        cols.append(wrap_idx(idx))                         # [128, CT*2]
    return np.concatenate(cols, axis=1)                    # [128, NCONF*2]


def build_gidx2(cfg: np.ndarray) -> np.ndarray:
    """Stage-2 indices per chunk: (c_local, j) -> c_local*32 + cfg[c,j]."""
    cols = []
    for ch in range(NCONF // CT):
        sl = cfg[ch * CT:(ch + 1) * CT]                    # [CT, 16]
        idx = (np.arange(CT)[:, None] * NE + sl).reshape(-1)
        cols.append(wrap_idx(idx))                         # [128, CT]
    return np.concatenate(cols, axis=1)                    # [128, NCONF]


def emit_program(nc, tc, aps, BCc: int):
    ctx = ExitStack()
    NBT = BCc // 128
    NCH = NCONF // CT          # chunks per b-tile
    NG = NCH // GC             # op-groups per b-tile
    aot, w32t, cirep, gidx1, gidx2, out = (
        aps["aot"], aps["w32t"], aps["cirep"], aps["gidx1"], aps["gidx2"],
        aps["out"])

    with ctx:
        cpool = ctx.enter_context(tc.tile_pool(name="consts", bufs=1))
        apool = ctx.enter_context(tc.tile_pool(name="aot", bufs=3))
        ps = ctx.enter_context(tc.tile_pool(name="ps", bufs=3, space="PSUM"))
        mst = ctx.enter_context(tc.tile_pool(name="mst", bufs=3))
        mem = ctx.enter_context(tc.tile_pool(name="mem", bufs=2))
        rp = ctx.enter_context(tc.tile_pool(name="rp", bufs=2))
        rtp = ctx.enter_context(tc.tile_pool(name="rtp", bufs=2))
        sg = ctx.enter_context(tc.tile_pool(name="sg", bufs=2))
        pb = ctx.enter_context(tc.tile_pool(name="pb", bufs=1))
        sm = ctx.enter_context(tc.tile_pool(name="sm", bufs=4))
        dets = ctx.enter_context(tc.tile_pool(name="dets", bufs=2))
        outp = ctx.enter_context(tc.tile_pool(name="outp", bufs=1))

        w32t_s = cpool.tile([128, NE], F32)
        cirep_s = cpool.tile([128, NCONF], F32)
        gidx1_s = cpool.tile([128, NCONF * 2], I16)
        gidx2_s = cpool.tile([128, NCONF], I16)
        nc.sync.dma_start(w32t_s[:], w32t[:])
        nc.sync.dma_start(cirep_s[:], cirep[:])
        nc.sync.dma_start(gidx1_s[:], gidx1[:])
        nc.sync.dma_start(gidx2_s[:], gidx2[:])

        out_sb = outp.tile([128, NBT], F32)

        # aot dram: [NAO, BCc*NE]
        aot3 = aot.rearrange("n (t f) -> t n f", f=128)    # 32*NBT tiles

        for bt in range(NBT):
            # ---------------- phase A: M_em [128b, (e, m32)] ------------
            m_em = mem.tile([128, NE * NE], F32)
            for t in range(NE):
                aot_t = apool.tile([128, 128], F32)
                nc.sync.dma_start(aot_t[:], aot3[bt * NE + t])
                mp = ps.tile([128, NE], F32)
                nc.tensor.matmul(mp[:], aot_t[:], w32t_s[:],
                                 start=True, stop=True)
                mst_t = mst.tile([128, NE], F32)
                nc.scalar.copy(mst_t[:], mp[:])
                # regroup [128(b,e), 32m] -> M_em[4 walkers, (e, m)]
                src = bass.AP(mst_t[:].tensor, mst_t[:].offset,
                              [[int(mst_t[:].ap[0][0]), 128], [1, NE]])
                dstv = m_em[:]
                dst = bass.AP(dstv.tensor,
                              dstv.offset + 4 * t * int(dstv.ap[0][0]),
                              [[int(dstv.ap[0][0]), 4], [NE, NE], [1, NE]])
                nc.sync.dma_start(dst, src)

            # ---------------- phase B: gathers + LU ---------------------
            dets_t = dets.tile([128, NCONF], F32)
            for grp in range(NG):
                sg_t = sg.tile([128, GC * CT * K * K], F32)
                for g in range(GC):
                    ch = grp * GC + g
                    r_t = rp.tile([128, CT * K * NE], F32)
                    nc.gpsimd.ap_gather(
                        r_t[:], m_em[:],
                        gidx1_s[:, ch * CT * 2:(ch + 1) * CT * 2],
                        channels=128, num_elems=NE * 2, d=16,
                        num_idxs=CT * K * 2)
                    rt_t = rtp.tile([128, CT * K * NE], F32)
                    rt_dst = bass.AP(
                        rt_t[:].tensor, rt_t[:].offset,
                        [[int(rt_t[:].ap[0][0]), 128],
                         [K * NE, CT], [K, NE], [1, K]])
                    r_src = bass.AP(
                        r_t[:].tensor, r_t[:].offset,
                        [[int(r_t[:].ap[0][0]), 128],
                         [K * NE, CT], [1, NE], [NE, K]])
                    nc.scalar.copy(rt_dst, r_src)
                    nc.gpsimd.ap_gather(
                        sg_t[:, g * CT * K * K:(g + 1) * CT * K * K],
                        rt_t[:],
                        gidx2_s[:, ch * CT:(ch + 1) * CT],
                        channels=128, num_elems=CT * NE, d=16,
                        num_idxs=CT * K)

                # ---- pivot-free LU over [g, c, j, i] ----
                S5 = sg_t[:].rearrange(
                    "p (g c j i) -> p g c j i", g=GC, c=CT, j=K)
                p_t = pb.tile([128, GC * CT * (K - 1) * (K - 1)], F32)
                P5 = p_t[:].rearrange(
                    "p (g c j i) -> p g c j i", g=GC, c=CT, j=K - 1)
                rec_t = sm.tile([128, GC * CT], F32, tag="rec")
                rec3 = rec_t[:].rearrange("p (g c) -> p g c", g=GC)
                rw_t = sm.tile([128, GC * CT * (K - 1)], F32, tag="rw")
                RW4 = rw_t[:].rearrange(
                    "p (g c i) -> p g c i", g=GC, c=CT)

                for k in range(K - 1):
                    r = K - 1 - k
                    piv = S5[:, :, :, k, k]
                    nc.vector.reciprocal(rec3, piv)
                    nc.vector.tensor_scalar(
                        rec_t[:], rec_t[:], -RCLAMP, RCLAMP,
                        op0=OP.max, op1=OP.min)
                    row = S5[:, :, :, k, k + 1:]
                    rwv = RW4[:, :, :, :r]
                    nc.vector.tensor_tensor(
                        rwv, row,
                        rec3.unsqueeze(3).broadcast_to([128, GC, CT, r]),
                        op=OP.mult)
                    col = S5[:, :, :, k + 1:, k]
                    Pv = P5[:, :, :, :r, :r]
                    nc.vector.tensor_tensor(
                        Pv,
                        col.unsqueeze(4).broadcast_to([128, GC, CT, r, r]),
                        rwv.unsqueeze(3).broadcast_to([128, GC, CT, r, r]),
                        op=OP.mult)
                    Sv = S5[:, :, :, k + 1:, k + 1:]
                    nc.vector.tensor_tensor(Sv, Sv, Pv, op=OP.subtract)

                # ---- det = prod(diag) via product tree ----
                pstr = int(sg_t[:].ap[0][0])
                base = sg_t[:].offset
                t8 = sm.tile([128, GC * CT * 8], F32, tag="t8")
                nc.vector.tensor_tensor(
                    t8[:].rearrange("p (g c x) -> p g c x", g=GC, c=CT),
                    bass.AP(sg_t[:].tensor, base,
                            [[pstr, 128], [CT * K * K, GC], [K * K, CT],
                             [34, 8]]),
                    bass.AP(sg_t[:].tensor, base + 17,
                            [[pstr, 128], [CT * K * K, GC], [K * K, CT],
                             [34, 8]]),
                    op=OP.mult)
                t4 = sm.tile([128, GC * CT * 4], F32, tag="t4")
                nc.vector.tensor_tensor(
                    t4[:].rearrange("p (g c x) -> p g c x", g=GC, c=CT),
                    bass.AP(t8[:].tensor, t8[:].offset,
                            [[int(t8[:].ap[0][0]), 128], [CT * 8, GC],
                             [8, CT], [2, 4]]),
                    bass.AP(t8[:].tensor, t8[:].offset + 1,
                            [[int(t8[:].ap[0][0]), 128], [CT * 8, GC],
                             [8, CT], [2, 4]]),
                    op=OP.mult)
                t2 = sm.tile([128, GC * CT * 2], F32, tag="t2")
                nc.vector.tensor_tensor(
                    t2[:].rearrange("p (g c x) -> p g c x", g=GC, c=CT),
                    bass.AP(t4[:].tensor, t4[:].offset,
                            [[int(t4[:].ap[0][0]), 128], [CT * 4, GC],
                             [4, CT], [2, 2]]),
                    bass.AP(t4[:].tensor, t4[:].offset + 1,
                            [[int(t4[:].ap[0][0]), 128], [CT * 4, GC],
                             [4, CT], [2, 2]]),
                    op=OP.mult)
                nc.vector.tensor_tensor(
                    dets_t[:, grp * GC * CT:(grp + 1) * GC * CT],
                    bass.AP(t2[:].tensor, t2[:].offset,
                            [[int(t2[:].ap[0][0]), 128], [2, GC * CT]]),
                    bass.AP(t2[:].tensor, t2[:].offset + 1,
                            [[int(t2[:].ap[0][0]), 128], [2, GC * CT]]),
                    op=OP.mult)

            wd = sm.tile([128, NCONF], F32, tag="wd")
            nc.vector.tensor_tensor(wd[:], dets_t[:], cirep_s[:], op=OP.mult)
            nc.vector.tensor_reduce(
                out_sb[:, bt:bt + 1], wd[:], axis=AX.X, op=OP.add)

        nc.sync.dma_start(out[:], out_sb[:])


def build(BCc: int):
    nc = bacc.Bacc("TRN2", target_bir_lowering=False, debug=False)
    aps = {}
    aps["aot"] = nc.dram_tensor(
        "aot", [NAO, BCc * NE], F32, kind="ExternalInput").ap()
    aps["w32t"] = nc.dram_tensor(
        "w32t", [NAO, NE], F32, kind="ExternalInput").ap()
    aps["cirep"] = nc.dram_tensor(
        "cirep", [128, NCONF], F32, kind="ExternalInput").ap()
    aps["gidx1"] = nc.dram_tensor(
        "gidx1", [128, NCONF * 2], I16, kind="ExternalInput").ap()
    aps["gidx2"] = nc.dram_tensor(
        "gidx2", [128, NCONF], I16, kind="ExternalInput").ap()
    aps["out"] = nc.dram_tensor(
        "out", [128, BCc // 128], F32, kind="ExternalOutput").ap()

    with tile.TileContext(nc) as tc:
        emit_program(nc, tc, aps, BCc)
    nc.compile()
    return nc


def host_inputs(ao_shard, mo_weight, ci_weight, configs):
    BCc = ao_shard.shape[0]
    w32 = mo_weight[:NE, :]
    return {
        "aot": np.ascontiguousarray(
            ao_shard.reshape(BCc * NE, NAO).T).astype(np.float32),
        "w32t": np.ascontiguousarray(w32.T).astype(np.float32),
        "cirep": np.ascontiguousarray(
            np.tile(ci_weight.astype(np.float32), (128, 1))),
        "gidx1": build_gidx1(configs),
        "gidx2": build_gidx2(configs),
    }


_CACHE: dict = {}


def _get_program():
    key = ("prog", BC, CT, GC)
    if key not in _CACHE:
        _CACHE[key] = build(BC)
    return _CACHE[key]


def kernel(ao, mo_weight, ci_weight, configs):
    ao = np.asarray(ao, dtype=np.float32)
    mo_weight = np.asarray(mo_weight, dtype=np.float32)
    ci_weight = np.asarray(ci_weight, dtype=np.float32)
    configs = np.asarray(configs, dtype=np.int32)
    assert ao.shape == (B, NE, NAO)

    nc = _get_program()
    in_maps = [
        host_inputs(ao[c * BC:(c + 1) * BC], mo_weight, ci_weight, configs)
        for c in range(NCORES)
    ]
    res = run_bass_kernel_spmd(nc, in_maps, core_ids=list(range(NCORES)))
    outs = []
    for c in range(NCORES):
        o = np.asarray(res.results[c]["out"])      # [128, NBT]
        outs.append(o.T.reshape(-1))               # b = bt*128 + p
    return np.concatenate(outs).astype(np.float32)[:, None]


def ref_algo(ao_shard, mo_weight, ci_weight, configs):
    """Numpy replica of the on-device algorithm (dev checking only)."""
    M = np.einsum("ben,mn->bem", ao_shard, mo_weight[:NE]).astype(np.float32)
    sub = M[:, configs[:, :, None], configs[:, None, :]].astype(np.float32)
    subT = np.swapaxes(sub, -1, -2)
    Bs = subT.shape[0]
    A = subT.reshape(-1, K, K).copy()
    rcl = np.float32(RCLAMP)
    for k in range(K - 1):
        piv = A[:, k, k].copy()
        with np.errstate(divide="ignore"):
            rec = (np.float32(1.0) / piv).astype(np.float32)
        rec = np.clip(rec, -rcl, rcl)
        rw = (A[:, k, k + 1:] * rec[:, None]).astype(np.float32)
        A[:, k + 1:, k + 1:] -= (
            A[:, k + 1:, k][:, :, None] * rw[:, None, :]).astype(np.float32)
    diag = A[:, np.arange(K), np.arange(K)]
    t8 = diag[:, 0::2] * diag[:, 1::2]
    t4 = t8[:, 0::2] * t8[:, 1::2]
    t2 = t4[:, 0::2] * t4[:, 1::2]
    det = (t2[:, 0] * t2[:, 1]).astype(np.float32)
    dets_ = det.reshape(Bs, NCONF)
    return (dets_ @ ci_weight.T.astype(np.float32)).astype(np.float32)


# revision 4
# speedup vs baseline: 1.0691x; 1.0691x over previous
"""Trainium2 Bass kernel for nn_NEURAL_PYSCF_WF (neural wavefunction).

reference:
  mo   = einsum('ben,mn->bem', ao, mo_weight)          # [B, 32, 128]
  sub  = mo[:, cfg[:,:,None], cfg[:,None,:]]           # [B, 128, 16, 16]
  dets = det(sub)                                      # [B, 128]
  out  = dets @ ci_weight.T                            # [B, 1]

Config indices are < 32, so only mo[:, :, :32] matters.

V2 strategy (8 NeuronCores, data-parallel over B=8192). Per core (1024
rows = 8 b-tiles of 128 walker-partitions):
  phase A (per b-tile): host supplies ao^T [nao, bc*ne]; 32 tiles of
    128 be-rows each: matmul with the ao^T tile as the stationary
    operand (lhsT=[n, be], rhs=w32t [n, 32]) -> PSUM [be, 32m] -> ACT
    copy -> SBUF -> SBUF->SBUF DMA regroup into M_em [128b, (e, m32)].
    No PE pre-transpose, no DRAM scratch roundtrip.
  phase B (per b-tile, chunks of CT=8 configs):
    g1 (GPSIMD ap_gather d=16): config rows -> R [c, i, m32]
    tr (ACT strided copy):      R -> Rt [c, m32, i16]
    g2 (GPSIMD ap_gather d=16): config cols -> S [c, j, i] written
       into a 4-chunk group tile (32 configs / DVE op via 4D APs)
    pivot-free LU on DVE over [g, c, j, i]; reciprocal clamped to
    +-1e6; det = prod(diag) via product tree.
  out[b] = sum_c ci[c] * det[b, c]  (TT mult + reduce).
"""

from contextlib import ExitStack

import numpy as np

import concourse.bass as bass
import concourse.bacc as bacc
import concourse.mybir as mybir
import concourse.tile as tile
from concourse.bass_utils import run_bass_kernel_spmd

F32 = mybir.dt.float32
I16 = mybir.dt.int16
AX = mybir.AxisListType
OP = mybir.AluOpType

B = 8192
NE = 32      # electrons (and the max config index)
NAO = 128
K = 16       # config size
NCONF = 128
NCORES = 8
BC = B // NCORES
RCLAMP = 1e6
CT = 8       # configs per gather chunk
GC = 4       # chunks per LU op-group (GC*CT = 32 configs per DVE op)


def wrap_idx(idx: np.ndarray) -> np.ndarray:
    """Wrap a flat index list into ap_gather's [128, n/16] layout."""
    n = idx.shape[0]
    assert n % 16 == 0
    w = idx.reshape(n // 16, 16).T.astype(np.int16)
    return np.tile(w, (8, 1))


def build_gidx1(cfg: np.ndarray) -> np.ndarray:
    """Stage-1 indices per chunk: (c, i, h) -> cfg[c,i]*2 + h (d=16)."""
    cols = []
    for ch in range(NCONF // CT):
        sl = cfg[ch * CT:(ch + 1) * CT]                    # [CT, 16]
        idx = (sl[:, :, None].astype(np.int64) * 2
               + np.arange(2)[None, None, :]).reshape(-1)  # CT*K*2
        cols.append(wrap_idx(idx))                         # [128, CT*2]
    return np.concatenate(cols, axis=1)                    # [128, NCONF*2]


def build_gidx2(cfg: np.ndarray) -> np.ndarray:
    """Stage-2 indices per chunk: (c_local, j) -> c_local*32 + cfg[c,j]."""
    cols = []
    for ch in range(NCONF // CT):
        sl = cfg[ch * CT:(ch + 1) * CT]                    # [CT, 16]
        idx = (np.arange(CT)[:, None] * NE + sl).reshape(-1)
        cols.append(wrap_idx(idx))                         # [128, CT]
    return np.concatenate(cols, axis=1)                    # [128, NCONF]


def emit_program(nc, tc, aps, BCc: int):
    ctx = ExitStack()
    NBT = BCc // 128
    NCH = NCONF // CT          # chunks per b-tile
    NG = NCH // GC             # op-groups per b-tile
    aot, w32t, cirep, gidx1, gidx2, out = (
        aps["aot"], aps["w32t"], aps["cirep"], aps["gidx1"], aps["gidx2"],
        aps["out"])

    with ctx:
        cpool = ctx.enter_context(tc.tile_pool(name="consts", bufs=1))
        apool = ctx.enter_context(tc.tile_pool(name="aot", bufs=3))
        ps = ctx.enter_context(tc.tile_pool(name="ps", bufs=3, space="PSUM"))
        mst = ctx.enter_context(tc.tile_pool(name="mst", bufs=3))
        mem = ctx.enter_context(tc.tile_pool(name="mem", bufs=2))
        rp = ctx.enter_context(tc.tile_pool(name="rp", bufs=1))
        rtp = ctx.enter_context(tc.tile_pool(name="rtp", bufs=2))
        sg = ctx.enter_context(tc.tile_pool(name="sg", bufs=2))
        pb = ctx.enter_context(tc.tile_pool(name="pb", bufs=2))
        sm = ctx.enter_context(tc.tile_pool(name="sm", bufs=4))
        dets = ctx.enter_context(tc.tile_pool(name="dets", bufs=2))
        outp = ctx.enter_context(tc.tile_pool(name="outp", bufs=1))

        w32t_s = cpool.tile([128, NE], F32)
        cirep_s = cpool.tile([128, NCONF], F32)
        gidx1_s = cpool.tile([128, NCONF * 2], I16)
        gidx2_s = cpool.tile([128, NCONF], I16)
        nc.sync.dma_start(w32t_s[:], w32t[:])
        nc.sync.dma_start(cirep_s[:], cirep[:])
        nc.sync.dma_start(gidx1_s[:], gidx1[:])
        nc.sync.dma_start(gidx2_s[:], gidx2[:])

        out_sb = outp.tile([128, NBT], F32)

        # aot dram: [NAO, BCc*NE]
        aot3 = aot.rearrange("n (t f) -> t n f", f=128)    # 32*NBT tiles

        for bt in range(NBT):
            # ---------------- phase A: M_em [128b, (e, m32)] ------------
            m_em = mem.tile([128, NE * NE], F32)
            for t in range(NE):
                aot_t = apool.tile([128, 128], F32)
                nc.sync.dma_start(aot_t[:], aot3[bt * NE + t])
                mp = ps.tile([128, NE], F32)
                nc.tensor.matmul(mp[:], aot_t[:], w32t_s[:],
                                 start=True, stop=True)
                mst_t = mst.tile([128, NE], F32)
                nc.scalar.copy(mst_t[:], mp[:])
                # regroup [128(b,e), 32m] -> M_em[4 walkers, (e, m)]
                src = bass.AP(mst_t[:].tensor, mst_t[:].offset,
                              [[int(mst_t[:].ap[0][0]), 128], [1, NE]])
                dstv = m_em[:]
                dst = bass.AP(dstv.tensor,
                              dstv.offset + 4 * t * int(dstv.ap[0][0]),
                              [[int(dstv.ap[0][0]), 4], [NE, NE], [1, NE]])
                nc.sync.dma_start(dst, src)

            # ---------------- phase B: gathers + LU ---------------------
            dets_t = dets.tile([128, NCONF], F32)
            for grp in range(NG):
                sg_t = sg.tile([128, GC * CT * K * K], F32)
                for g in range(GC):
                    ch = grp * GC + g
                    r_t = rp.tile([128, CT * K * NE], F32)
                    nc.gpsimd.ap_gather(
                        r_t[:], m_em[:],
                        gidx1_s[:, ch * CT * 2:(ch + 1) * CT * 2],
                        channels=128, num_elems=NE * 2, d=16,
                        num_idxs=CT * K * 2)
                    rt_t = rtp.tile([128, CT * K * NE], F32)
                    rt_dst = bass.AP(
                        rt_t[:].tensor, rt_t[:].offset,
                        [[int(rt_t[:].ap[0][0]), 128],
                         [K * NE, CT], [K, NE], [1, K]])
                    r_src = bass.AP(
                        r_t[:].tensor, r_t[:].offset,
                        [[int(r_t[:].ap[0][0]), 128],
                         [K * NE, CT], [1, NE], [NE, K]])
                    if ch % 8 < 5:
                        nc.vector.tensor_copy(rt_dst, r_src)
                    else:
                        nc.scalar.copy(rt_dst, r_src)
                    nc.gpsimd.ap_gather(
                        sg_t[:, g * CT * K * K:(g + 1) * CT * K * K],
                        rt_t[:],
                        gidx2_s[:, ch * CT:(ch + 1) * CT],
                        channels=128, num_elems=CT * NE, d=16,
                        num_idxs=CT * K)

                # ---- pivot-free LU over [g, c, j, i] ----
                S5 = sg_t[:].rearrange(
                    "p (g c j i) -> p g c j i", g=GC, c=CT, j=K)
                p_t = pb.tile([128, GC * CT * (K - 1) * (K - 1)], F32)
                P5 = p_t[:].rearrange(
                    "p (g c j i) -> p g c j i", g=GC, c=CT, j=K - 1)
                rec_t = sm.tile([128, GC * CT], F32, tag="rec")
                rec3 = rec_t[:].rearrange("p (g c) -> p g c", g=GC)
                rw_t = sm.tile([128, GC * CT * (K - 1)], F32, tag="rw")
                RW4 = rw_t[:].rearrange(
                    "p (g c i) -> p g c i", g=GC, c=CT)

                for k in range(K - 1):
                    r = K - 1 - k
                    piv = S5[:, :, :, k, k]
                    nc.vector.reciprocal(rec3, piv)
                    nc.vector.tensor_scalar(
                        rec_t[:], rec_t[:], -RCLAMP, RCLAMP,
                        op0=OP.max, op1=OP.min)
                    row = S5[:, :, :, k, k + 1:]
                    rwv = RW4[:, :, :, :r]
                    nc.vector.tensor_tensor(
                        rwv, row,
                        rec3.unsqueeze(3).broadcast_to([128, GC, CT, r]),
                        op=OP.mult)
                    col = S5[:, :, :, k + 1:, k]
                    Pv = P5[:, :, :, :r, :r]
                    nc.vector.tensor_tensor(
                        Pv,
                        col.unsqueeze(4).broadcast_to([128, GC, CT, r, r]),
                        rwv.unsqueeze(3).broadcast_to([128, GC, CT, r, r]),
                        op=OP.mult)
                    Sv = S5[:, :, :, k + 1:, k + 1:]
                    nc.vector.tensor_tensor(Sv, Sv, Pv, op=OP.subtract)

                # ---- det = prod(diag) via product tree ----
                pstr = int(sg_t[:].ap[0][0])
                base = sg_t[:].offset
                t8 = sm.tile([128, GC * CT * 8], F32, tag="t8")
                nc.vector.tensor_tensor(
                    t8[:].rearrange("p (g c x) -> p g c x", g=GC, c=CT),
                    bass.AP(sg_t[:].tensor, base,
                            [[pstr, 128], [CT * K * K, GC], [K * K, CT],
                             [34, 8]]),
                    bass.AP(sg_t[:].tensor, base + 17,
                            [[pstr, 128], [CT * K * K, GC], [K * K, CT],
                             [34, 8]]),
                    op=OP.mult)
                t4 = sm.tile([128, GC * CT * 4], F32, tag="t4")
                nc.vector.tensor_tensor(
                    t4[:].rearrange("p (g c x) -> p g c x", g=GC, c=CT),
                    bass.AP(t8[:].tensor, t8[:].offset,
                            [[int(t8[:].ap[0][0]), 128], [CT * 8, GC],
                             [8, CT], [2, 4]]),
                    bass.AP(t8[:].tensor, t8[:].offset + 1,
                            [[int(t8[:].ap[0][0]), 128], [CT * 8, GC],
                             [8, CT], [2, 4]]),
                    op=OP.mult)
                t2 = sm.tile([128, GC * CT * 2], F32, tag="t2")
                nc.vector.tensor_tensor(
                    t2[:].rearrange("p (g c x) -> p g c x", g=GC, c=CT),
                    bass.AP(t4[:].tensor, t4[:].offset,
                            [[int(t4[:].ap[0][0]), 128], [CT * 4, GC],
                             [4, CT], [2, 2]]),
                    bass.AP(t4[:].tensor, t4[:].offset + 1,
                            [[int(t4[:].ap[0][0]), 128], [CT * 4, GC],
                             [4, CT], [2, 2]]),
                    op=OP.mult)
                nc.vector.tensor_tensor(
                    dets_t[:, grp * GC * CT:(grp + 1) * GC * CT],
                    bass.AP(t2[:].tensor, t2[:].offset,
                            [[int(t2[:].ap[0][0]), 128], [2, GC * CT]]),
                    bass.AP(t2[:].tensor, t2[:].offset + 1,
                            [[int(t2[:].ap[0][0]), 128], [2, GC * CT]]),
                    op=OP.mult)

            wd = sm.tile([128, NCONF], F32, tag="wd")
            nc.vector.tensor_tensor(wd[:], dets_t[:], cirep_s[:], op=OP.mult)
            nc.vector.tensor_reduce(
                out_sb[:, bt:bt + 1], wd[:], axis=AX.X, op=OP.add)

        nc.sync.dma_start(out[:], out_sb[:])


def build(BCc: int):
    nc = bacc.Bacc("TRN2", target_bir_lowering=False, debug=False)
    aps = {}
    aps["aot"] = nc.dram_tensor(
        "aot", [NAO, BCc * NE], F32, kind="ExternalInput").ap()
    aps["w32t"] = nc.dram_tensor(
        "w32t", [NAO, NE], F32, kind="ExternalInput").ap()
    aps["cirep"] = nc.dram_tensor(
        "cirep", [128, NCONF], F32, kind="ExternalInput").ap()
    aps["gidx1"] = nc.dram_tensor(
        "gidx1", [128, NCONF * 2], I16, kind="ExternalInput").ap()
    aps["gidx2"] = nc.dram_tensor(
        "gidx2", [128, NCONF], I16, kind="ExternalInput").ap()
    aps["out"] = nc.dram_tensor(
        "out", [128, BCc // 128], F32, kind="ExternalOutput").ap()

    with tile.TileContext(nc) as tc:
        emit_program(nc, tc, aps, BCc)
    nc.compile()
    return nc


def host_inputs(ao_shard, mo_weight, ci_weight, configs):
    BCc = ao_shard.shape[0]
    w32 = mo_weight[:NE, :]
    return {
        "aot": np.ascontiguousarray(
            ao_shard.reshape(BCc * NE, NAO).T).astype(np.float32),
        "w32t": np.ascontiguousarray(w32.T).astype(np.float32),
        "cirep": np.ascontiguousarray(
            np.tile(ci_weight.astype(np.float32), (128, 1))),
        "gidx1": build_gidx1(configs),
        "gidx2": build_gidx2(configs),
    }


_CACHE: dict = {}


def _get_program():
    key = ("prog", BC, CT, GC)
    if key not in _CACHE:
        _CACHE[key] = build(BC)
    return _CACHE[key]


def kernel(ao, mo_weight, ci_weight, configs):
    ao = np.asarray(ao, dtype=np.float32)
    mo_weight = np.asarray(mo_weight, dtype=np.float32)
    ci_weight = np.asarray(ci_weight, dtype=np.float32)
    configs = np.asarray(configs, dtype=np.int32)
    assert ao.shape == (B, NE, NAO)

    nc = _get_program()
    in_maps = [
        host_inputs(ao[c * BC:(c + 1) * BC], mo_weight, ci_weight, configs)
        for c in range(NCORES)
    ]
    res = run_bass_kernel_spmd(nc, in_maps, core_ids=list(range(NCORES)))
    outs = []
    for c in range(NCORES):
        o = np.asarray(res.results[c]["out"])      # [128, NBT]
        outs.append(o.T.reshape(-1))               # b = bt*128 + p
    return np.concatenate(outs).astype(np.float32)[:, None]


def ref_algo(ao_shard, mo_weight, ci_weight, configs):
    """Numpy replica of the on-device algorithm (dev checking only)."""
    M = np.einsum("ben,mn->bem", ao_shard, mo_weight[:NE]).astype(np.float32)
    sub = M[:, configs[:, :, None], configs[:, None, :]].astype(np.float32)
    subT = np.swapaxes(sub, -1, -2)
    Bs = subT.shape[0]
    A = subT.reshape(-1, K, K).copy()
    rcl = np.float32(RCLAMP)
    for k in range(K - 1):
        piv = A[:, k, k].copy()
        with np.errstate(divide="ignore"):
            rec = (np.float32(1.0) / piv).astype(np.float32)
        rec = np.clip(rec, -rcl, rcl)
        rw = (A[:, k, k + 1:] * rec[:, None]).astype(np.float32)
        A[:, k + 1:, k + 1:] -= (
            A[:, k + 1:, k][:, :, None] * rw[:, None, :]).astype(np.float32)
    diag = A[:, np.arange(K), np.arange(K)]
    t8 = diag[:, 0::2] * diag[:, 1::2]
    t4 = t8[:, 0::2] * t8[:, 1::2]
    t2 = t4[:, 0::2] * t4[:, 1::2]
    det = (t2[:, 0] * t2[:, 1]).astype(np.float32)
    dets_ = det.reshape(Bs, NCONF)
    return (dets_ @ ci_weight.T.astype(np.float32)).astype(np.float32)
